# revision 50
# baseline (speedup 1.0000x reference)
"""GAT (2-layer) Trainium2 Bass kernel — streamed-edge, engine-balanced.

Strategy (8 NeuronCores, SPMD, 3 launches):
  - Destination-sharded edge parallelism: core k owns dst nodes
    [12500k, 12500(k+1)), degree-sorted and packed into ELL tiles
    [128 dst x K_t slots] (K_t padded to a multiple of 4).
  - Per-edge source payloads are assembled on the host (pure data
    movement between launches) and DMA-streamed sequentially; no
    device-side gather.
      launch A: per-node transform  [h | a_src | a_dst] = x @ W1aug
      launch B: layer-1 edge pass (softmax + aggregate + ELU + @W2aug)
      launch C: layer-2 edge pass -> final output rows
  - Engine balance in the edge passes: message multiply on gpsimd,
    tree-fold + reduce on DVE, exp/ELU/PSUM-copies on the scalar
    engine, h2@W2aug on the tensor engine.
  - Layer-1 payload rows are [h(64) | ones(8) | a_src(8)] so the
    softmax denominator falls out of the same fold/reduce tree as the
    numerator (features 64:72 of the weighted message are the weights).
  - Layer-2 uses exp(lrelu(x)) == max(exp(x), exp(0.2x)) so the weight
    needs no lrelu op, and processes tiles in pairs (pair-equalized K,
    host-interleaved stream) to amortize per-op overheads; layer-1
    finalize (ELU + @W2aug) is likewise paired, with one 128x128
    transpose and a block-diagonal W2 matmul per pair.
  - Padded ELL slots carry a_src = ASENT so their weight is exactly 0.

kernel(**inputs) -> np.ndarray [100000, 16] float32.
"""
import sys

sys.path.insert(0, "/opt/trn_rl_repo")

import numpy as np
import concourse.bass as bass
import concourse.bacc as bacc
import concourse.tile as tile
from concourse import mybir
from concourse.bass_utils import run_bass_kernel_spmd

AP = bass.AP
F32 = mybir.dt.float32
AF = mybir.ActivationFunctionType
ALU = mybir.AluOpType
AX = mybir.AxisListType

# Problem constants (hardcoded per the harness contract).
N = 100000
E = 1600000
IN_C = 128
HID = 8
HEADS = 8
C1 = HEADS * HID          # 64
OUT_C = 16
NEG_SLOPE = 0.2
NCORES = 8

NLOC = N // NCORES        # 12500 local dst nodes per core
NT = 98                   # node tiles of 128 (98*128 = 12544)
NL = NT * 128             # 12544 padded local nodes
P1 = 80                   # layer-1 payload: h(64) | ones(8) | a_src(8)
P2 = 17                   # layer-2 payload: h2W2(16) | a_src2(1)
ASENT = -30000.0          # sentinel a_src; weight becomes exactly 0
EPS = 1e-16

_cache = {}


# --------------------------------------------------------------------------
# Host-side preprocessing (pure data movement + O(F^2) weight prep)
# --------------------------------------------------------------------------
def _prep(x, edge_index, W1, att_src1, att_dst1, W2, att_src2, att_dst2):
    src = edge_index[0].astype(np.int64)
    dst = edge_index[1].astype(np.int64)

    W1r = W1.reshape(IN_C, HEADS, HID)
    v_src1 = np.einsum("khc,hc->kh", W1r, att_src1).astype(np.float32)
    v_dst1 = np.einsum("khc,hc->kh", W1r, att_dst1).astype(np.float32)
    # h is stored feature-major (column 8b+j = feat b, head j) so both the
    # h-blocks and the ones-block of the edge-pass multiply broadcast the
    # per-head weight along the innermost 8 elements.
    perm = np.arange(C1).reshape(HEADS, HID).T.reshape(-1)
    W1aug = np.concatenate(
        [W1[:, perm], v_src1, v_dst1], axis=1).astype(np.float32)
    v_src2 = (W2 @ att_src2[0]).astype(np.float32)
    v_dst2 = (W2 @ att_dst2[0]).astype(np.float32)
    W2aug = np.concatenate(
        [W2, v_src2[:, None], v_dst2[:, None]],
        axis=1).astype(np.float32)[perm, :]

    order = np.argsort(dst, kind="stable")
    deg = np.bincount(dst, minlength=N).astype(np.int64)
    cum = np.zeros(N + 1, dtype=np.int64)
    np.cumsum(deg, out=cum[1:])

    cores = []
    for k in range(NCORES):
        ids = np.arange(k * NLOC, (k + 1) * NLOC)
        dk = deg[ids]
        sp = np.argsort(-dk, kind="stable")
        cores.append((ids[sp], dk[sp]))

    # global K schedule, padded to a multiple of 4
    K = np.zeros(NT, dtype=np.int64)
    for k in range(NCORES):
        ds = np.zeros(NL, dtype=np.int64)
        ds[:NLOC] = cores[k][1]
        K = np.maximum(K, ds.reshape(NT, 128).max(axis=1))
    K = ((K + 3) // 4) * 4
    K = np.maximum(K, 4)
    Ks = tuple(int(v) for v in K)

    Kmax = int(K.max())
    per_core = []
    for k in range(NCORES):
        sorted_ids, deg_sorted = cores[k]
        dpad = np.zeros(NL, dtype=np.int64)
        dpad[:NLOC] = deg_sorted
        start = np.zeros(NL, dtype=np.int64)
        start[:NLOC] = cum[sorted_ids]
        colr = np.arange(Kmax)
        valid = colr[None, :] < dpad[:, None]              # [NL, Kmax]
        epos = start[:, None] + colr[None, :]
        srcs = np.zeros((NL, Kmax), dtype=np.int64)
        srcs[valid] = src[order[epos[valid]]]

        xlocT = np.zeros((IN_C, NL), dtype=np.float32)
        xlocT[:, :NLOC] = x[sorted_ids].T

        per_core.append(dict(srcs=srcs, valid=valid, xlocT=xlocT,
                             sorted_ids=sorted_ids))

    shared = dict(W1aug=W1aug, W2aug=W2aug, perm=perm)
    return shared, per_core, Ks


def _build_stream(Ks, srcs, valid, tab, width, sent_lo, sent_hi,
                  paired=False):
    """Per-edge payload stream: one [128, K_t, width] block per tile, or
    (paired) one [128, 2, K, width] block per tile pair."""
    def tile_pay(t, kt):
        rows = slice(t * 128, (t + 1) * 128)
        blk = srcs[rows, :kt]                           # [128, kt]
        pay = tab[blk.reshape(-1)].reshape(128, kt, width)
        iv = ~valid[rows, :kt]
        if iv.any():
            pay[iv, sent_lo:sent_hi] = ASENT
        return pay
    parts = []
    if paired:
        for m in range(len(Ks) // 2):
            kt = Ks[2 * m]
            pair = np.stack([tile_pay(2 * m, kt), tile_pay(2 * m + 1, kt)],
                            axis=1)                     # [128, 2, kt, w]
            parts.append(pair.reshape(-1))
    else:
        for t, kt in enumerate(Ks):
            parts.append(tile_pay(t, kt).reshape(-1))
    return np.concatenate(parts)


# --------------------------------------------------------------------------
# Launch A: per-node transform  t1T = [W1 | v_src1 | v_dst1]^T @ xloc
# --------------------------------------------------------------------------
def _build_tform():
    nc = bacc.Bacc("TRN2", target_bir_lowering=False, debug=False,
                   num_devices=NCORES)
    xlocT = nc.dram_tensor("xlocT", [IN_C, NL], F32, kind="ExternalInput").ap()
    W1aug = nc.dram_tensor("W1aug", [IN_C, P1], F32, kind="ExternalInput").ap()
    t1 = nc.dram_tensor("t1", [NL, P1], F32, kind="ExternalOutput").ap()

    with tile.TileContext(nc) as tc:
        with tc.tile_pool(name="const", bufs=1) as constp, \
             tc.tile_pool(name="lhs", bufs=3) as lhsp, \
             tc.tile_pool(name="ps", bufs=2, space="PSUM") as psp, \
             tc.tile_pool(name="cp", bufs=3) as cpp:
            w = constp.tile([IN_C, P1], F32)
            nc.sync.dma_start(out=w[:], in_=W1aug[:, :])
            for B in range((NL + 511) // 512):
                nb = min(4, (NL - 512 * B) // 128)
                lt = lhsp.tile([IN_C, 512], F32, tag="lhs")
                eng = nc.sync if B % 2 == 0 else nc.scalar
                eng.dma_start(
                    out=lt[:, :128 * nb],
                    in_=xlocT[:, 512 * B: 512 * B + 128 * nb])
                ps = psp.tile([128, 4, P1], F32)
                for j in range(nb):
                    nc.tensor.matmul(out=ps[:, j, :],
                                     lhsT=lt[:, 128 * j: 128 * (j + 1)],
                                     rhs=w[:], start=True, stop=True)
                cp = cpp.tile([128, 4, P1], F32, tag="cp")
                nc.scalar.activation(out=cp[:, :nb, :], in_=ps[:, :nb, :],
                                     func=AF.Copy)
                nc.sync.dma_start(
                    out=AP(tensor=t1.tensor, offset=512 * B * P1,
                           ap=[[P1, 128], [128 * P1, nb], [1, P1]]),
                    in_=cp[:, :nb, :])
    nc.compile()
    return nc


# --------------------------------------------------------------------------
# Launch B: layer-1 edge pass + finalize (ELU + @W2aug)
# --------------------------------------------------------------------------
def _build_l1(Ks, s1len, zb1):
    nc = bacc.Bacc("TRN2", target_bir_lowering=False, debug=False,
                   num_devices=NCORES)
    s1 = nc.dram_tensor("s1", [s1len], F32, kind="ExternalInput").ap()
    adT = nc.dram_tensor("adT", [128, NT * 8], F32, kind="ExternalInput").ap()
    b1 = nc.dram_tensor("b1", [128, C1], F32, kind="ExternalInput").ap()
    W2blk = nc.dram_tensor("W2blk", [128, 36], F32, kind="ExternalInput").ap()
    p2 = nc.dram_tensor("p2", [NL, 18], F32, kind="ExternalOutput").ap()

    from concourse.masks import make_identity
    KM = max(Ks)

    with tile.TileContext(nc) as tc:
        with tc.tile_pool(name="const", bufs=1) as constp, \
             tc.tile_pool(name="gp", bufs=5) as gp, \
             tc.tile_pool(name="ep", bufs=5) as ep, \
             tc.tile_pool(name="tp", bufs=5) as tp, \
             tc.tile_pool(name="fp", bufs=4) as fp, \
             tc.tile_pool(name="zp", bufs=4) as zp, \
             tc.tile_pool(name="psT", bufs=3, space="PSUM") as psTp, \
             tc.tile_pool(name="psP", bufs=3, space="PSUM") as psPp:

            adsb = constp.tile([128, NT * 8], F32)
            nc.sync.dma_start(out=adsb[:], in_=adT[:, :])
            b1sb = constp.tile([128, C1], F32)
            nc.sync.dma_start(out=b1sb[:], in_=b1[:, :])
            w2sb = constp.tile([128, 36], F32)
            nc.sync.dma_start(out=w2sb[:], in_=W2blk[:, :])
            ident = constp.tile([128, 128], F32)
            make_identity(nc, ident[:])
            epssb = constp.tile([128, 1], F32)
            nc.vector.memset(epssb[:], EPS)

            off = 0
            aggq = None
            for t in range(NT):
                kt = Ks[t]
                kh = kt // 2
                # one DMA per tile
                G = gp.tile([128, KM, P1], F32, tag="G")
                nc.sync.dma_start(
                    out=G[:, :kt, :],
                    in_=AP(tensor=s1.tensor, offset=off,
                           ap=[[kt * P1, 128], [1, kt * P1]]))
                off += 128 * kt * P1
                go = G[:, :kt, :].offset
                # w = exp(lrelu(a_src + a_dst))   [128, kt, 8]
                ea = ep.tile([128, KM, 8], F32, tag="ea")
                eav = ea[:, :kt, :]
                nc.gpsimd.tensor_tensor(
                    out=eav,
                    in0=AP(tensor=G.tensor, offset=go + 72,
                           ap=[G[:].ap[0], [P1, kt], [1, 8]]),
                    in1=AP(tensor=adsb.tensor,
                           offset=adsb[:].offset + 8 * t,
                           ap=[adsb[:].ap[0], [0, kt], [1, 8]]),
                    op=ALU.add)
                nc.vector.scalar_tensor_tensor(
                    out=eav, in0=eav, scalar=NEG_SLOPE, in1=eav,
                    op0=ALU.mult, op1=ALU.max)
                nc.scalar.activation(out=eav, in_=eav, func=AF.Exp)
                # T = [h|ones] * w  -> [128, kt, 9, 8]  (gpsimd)
                T = tp.tile([128, KM, 9, 8], F32, tag="T")
                Tv = T[:, :kt, :, :]
                nc.gpsimd.tensor_tensor(
                    out=Tv,
                    in0=AP(tensor=G.tensor, offset=go,
                           ap=[G[:].ap[0], [P1, kt], [8, 9], [1, 8]]),
                    in1=AP(tensor=ea.tensor, offset=eav.offset,
                           ap=[ea[:].ap[0], [8, kt], [0, 9], [1, 8]]),
                    op=ALU.mult)
                # fold + reduce over slots -> aggq[q] = [num(64) | den(8)]
                F = fp.tile([128, KM // 2, 9, 8], F32, tag="F")
                Fv = F[:, :kh, :, :]
                feng = nc.vector if t % 2 == 0 else nc.gpsimd
                feng.tensor_tensor(
                    out=Fv, in0=T[:, :kh, :, :], in1=T[:, kh:kt, :, :],
                    op=ALU.add)
                kq = kh // 2
                F2 = fp.tile([128, KM // 4, 9, 8], F32, tag="F2")
                F2v = F2[:, :kq, :, :]
                nc.vector.tensor_tensor(
                    out=F2v, in0=F[:, :kq, :, :], in1=F[:, kq:kh, :, :],
                    op=ALU.add)
                q = t % 2
                if q == 0:
                    aggq = zp.tile([128, 2, 72], F32, tag="agg")
                nc.vector.tensor_reduce(
                    out=aggq[:, q, :],
                    in_=AP(tensor=F2.tensor, offset=F2v.offset,
                           ap=[F2[:].ap[0], [1, 72], [72, kq]]),
                    axis=AX.X, op=ALU.add)
                if q == 0:
                    continue
                # finalize the pair (t-1, t)
                den = zp.tile([128, 2, 8], F32, tag="den")
                nc.scalar.activation(
                    out=den[:],
                    in_=AP(tensor=aggq.tensor, offset=aggq[:].offset + 64,
                           ap=[aggq[:].ap[0], [72, 2], [1, 8]]),
                    func=AF.Identity, bias=epssb[:, 0:1])
                rec = zp.tile([128, 2, 8], F32, tag="rec")
                nc.vector.reciprocal(rec[:], den[:])
                out1 = zp.tile([128, 2, C1], F32, tag="out1")
                nc.vector.tensor_tensor(
                    out=out1[:],
                    in0=AP(tensor=aggq.tensor, offset=aggq[:].offset,
                           ap=[aggq[:].ap[0], [72, 2], [1, 64]]),
                    in1=AP(tensor=rec.tensor, offset=rec[:].offset,
                           ap=[rec[:].ap[0], [8, 2], [0, 8], [1, 8]]),
                    op=ALU.mult)
                if not zb1:
                    nc.vector.tensor_tensor(
                        out=out1[:], in0=out1[:],
                        in1=AP(tensor=b1sb.tensor, offset=b1sb[:].offset,
                               ap=[b1sb[:].ap[0], [0, 2], [1, C1]]),
                        op=ALU.add)
                # elu = relu(x) + exp(min(x,0)) - 1
                r = zp.tile([128, 2, C1], F32, tag="relu")
                nc.scalar.activation(out=r[:], in_=out1[:], func=AF.Relu)
                mn = zp.tile([128, 2, C1], F32, tag="mn")
                nc.scalar.activation(out=mn[:], in_=out1[:], func=AF.Relu,
                                     scale=-1.0)
                nc.scalar.activation(out=mn[:], in_=mn[:], func=AF.Exp,
                                     scale=-1.0)
                h2 = zp.tile([128, 2, C1], F32, tag="h2")
                nc.vector.scalar_tensor_tensor(
                    out=h2[:], in0=r[:], scalar=-1.0, in1=mn[:],
                    op0=ALU.add, op1=ALU.add)
                # p2 pair = h2 @ blockdiag(W2aug, W2aug); one transpose
                pst = psTp.tile([128, 128], F32)
                nc.tensor.transpose(
                    out=pst[:],
                    in_=AP(tensor=h2.tensor, offset=h2[:].offset,
                           ap=[h2[:].ap[0], [1, 128]]),
                    identity=ident[:])
                h2T = zp.tile([128, 128], F32, tag="h2T")
                nc.scalar.activation(out=h2T[:], in_=pst[:], func=AF.Copy)
                psp = psPp.tile([128, 2, 18], F32)
                nc.tensor.matmul(out=psp[:], lhsT=h2T[:], rhs=w2sb[:],
                                 start=True, stop=True)
                p2sb = zp.tile([128, 2, 18], F32, tag="p2sb")
                nc.scalar.activation(out=p2sb[:], in_=psp[:], func=AF.Copy)
                nc.sync.dma_start(
                    out=AP(tensor=p2.tensor, offset=(t - 1) * 128 * 18,
                           ap=[[18, 128], [128 * 18, 2], [1, 18]]),
                    in_=p2sb[:])
    nc.compile()
    return nc


# --------------------------------------------------------------------------
# Launch C: layer-2 edge pass
# --------------------------------------------------------------------------
def _build_l2(Ks2, s2len, zb2):
    nc = bacc.Bacc("TRN2", target_bir_lowering=False, debug=False,
                   num_devices=NCORES)
    s2 = nc.dram_tensor("s2", [s2len], F32, kind="ExternalInput").ap()
    ad2 = nc.dram_tensor("ad2", [128, NT], F32, kind="ExternalInput").ap()
    b2 = nc.dram_tensor("b2", [128, OUT_C], F32, kind="ExternalInput").ap()
    out2 = nc.dram_tensor("out2", [NL, OUT_C], F32, kind="ExternalOutput").ap()
    KM = max(Ks2)

    with tile.TileContext(nc) as tc:
        with tc.tile_pool(name="const", bufs=1) as constp, \
             tc.tile_pool(name="gp", bufs=5) as gp, \
             tc.tile_pool(name="ep", bufs=4) as ep, \
             tc.tile_pool(name="tp", bufs=4) as tp, \
             tc.tile_pool(name="zp", bufs=4) as zp:

            adsb = constp.tile([128, NT], F32)
            nc.sync.dma_start(out=adsb[:], in_=ad2[:, :])
            b2sb = constp.tile([128, OUT_C], F32)
            nc.sync.dma_start(out=b2sb[:], in_=b2[:, :])
            epssb = constp.tile([128, 1], F32)
            nc.vector.memset(epssb[:], EPS)

            off = 0
            for m in range(NT // 2):
                kt = Ks2[2 * m]
                # one DMA per tile pair (both tiles share kt)
                G = gp.tile([128, 2, KM, P2], F32, tag="G")
                Gv = G[:, :, :kt, :]
                eng = nc.sync if m % 2 == 0 else nc.scalar
                eng.dma_start(
                    out=Gv,
                    in_=AP(tensor=s2.tensor, offset=off,
                           ap=[[2 * kt * P2, 128], [1, 2 * kt * P2]]))
                off += 128 * 2 * kt * P2
                go = Gv.offset
                gq = KM * P2          # q stride inside G
                # e = a_src + a_dst   [128, 2, kt]
                e = ep.tile([128, 2, KM], F32, tag="e")
                ev = e[:, :, :kt]
                nc.vector.tensor_tensor(
                    out=ev,
                    in0=AP(tensor=G.tensor, offset=go + 16,
                           ap=[G[:].ap[0], [gq, 2], [P2, kt]]),
                    in1=AP(tensor=adsb.tensor,
                           offset=adsb[:].offset + 2 * m,
                           ap=[adsb[:].ap[0], [1, 2], [0, kt]]),
                    op=ALU.add)
                # w = exp(lrelu(e)) = max(exp(e), exp(0.2 e))
                e2 = ep.tile([128, 2, KM], F32, tag="e2")
                e2v = e2[:, :, :kt]
                nc.scalar.activation(out=e2v, in_=ev, func=AF.Exp,
                                     scale=NEG_SLOPE)
                nc.scalar.activation(out=ev, in_=ev, func=AF.Exp)
                w = ep.tile([128, 2, KM], F32, tag="w")
                wv = w[:, :, :kt]
                nc.vector.tensor_tensor(out=wv, in0=ev, in1=e2v, op=ALU.max)
                den = zp.tile([128, 2], F32, tag="den")
                nc.vector.tensor_reduce(
                    out=den[:],
                    in_=AP(tensor=w.tensor, offset=wv.offset,
                           ap=[w[:].ap[0], [KM, 2], [1, kt]]),
                    axis=AX.X, op=ALU.add)
                rec = zp.tile([128, 2], F32, tag="rec")
                nc.scalar.activation(out=den[:], in_=den[:],
                                     func=AF.Identity, bias=epssb[:, 0:1])
                nc.vector.reciprocal(rec[:], den[:])
                # T = h2w2 * w   (gpsimd)
                T = tp.tile([128, 2, KM, OUT_C], F32, tag="T")
                Tv = T[:, :, :kt, :]
                nc.gpsimd.tensor_tensor(
                    out=Tv,
                    in0=AP(tensor=G.tensor, offset=go,
                           ap=[G[:].ap[0], [gq, 2], [P2, kt], [1, OUT_C]]),
                    in1=AP(tensor=w.tensor, offset=wv.offset,
                           ap=[w[:].ap[0], [KM, 2], [1, kt], [0, OUT_C]]),
                    op=ALU.mult)
                num = zp.tile([128, 2, OUT_C], F32, tag="num")
                nc.vector.tensor_reduce(
                    out=num[:],
                    in_=AP(tensor=T.tensor, offset=Tv.offset,
                           ap=[T[:].ap[0], [KM * OUT_C, 2], [1, OUT_C],
                               [OUT_C, kt]]),
                    axis=AX.X, op=ALU.add)
                o = zp.tile([128, 2, OUT_C], F32, tag="o")
                nc.vector.tensor_tensor(
                    out=o[:], in0=num[:],
                    in1=AP(tensor=rec.tensor, offset=rec[:].offset,
                           ap=[rec[:].ap[0], [1, 2], [0, OUT_C]]),
                    op=ALU.mult)
                if not zb2:
                    nc.vector.tensor_tensor(
                        out=o[:], in0=o[:],
                        in1=AP(tensor=b2sb.tensor, offset=b2sb[:].offset,
                               ap=[b2sb[:].ap[0], [0, 2], [1, OUT_C]]),
                        op=ALU.add)
                nc.sync.dma_start(
                    out=AP(tensor=out2.tensor, offset=2 * m * 128 * OUT_C,
                           ap=[[OUT_C, 128], [128 * OUT_C, 2], [1, OUT_C]]),
                    in_=o[:])
    nc.compile()
    return nc


# --------------------------------------------------------------------------
# Entry point
# --------------------------------------------------------------------------
TRACE = False
LAST_EXEC_NS = []


def _run_retry(nc, in_maps, core_ids, trace):
    import time as _time
    last = None
    for attempt in range(3):
        try:
            return run_bass_kernel_spmd(nc, in_maps, core_ids, trace=trace)
        except Exception as e:  # transient NRT_EXEC_UNIT_UNRECOVERABLE
            last = e
            _time.sleep(10)
    raise last


def kernel(x, edge_index, W1, b1, att_src1, att_dst1, W2, b2, att_src2,
           att_dst2):
    global LAST_EXEC_NS
    LAST_EXEC_NS = []
    x = np.asarray(x, dtype=np.float32)
    edge_index = np.asarray(edge_index)
    shared, per_core, Ks = _prep(
        x, edge_index, np.asarray(W1), np.asarray(att_src1),
        np.asarray(att_dst1), np.asarray(W2), np.asarray(att_src2),
        np.asarray(att_dst2))

    s1len = 128 * sum(Ks) * P1
    Ks2 = tuple(max(Ks[2 * m], Ks[2 * m + 1]) for m in range(NT // 2)
                for _ in range(2))
    s2len = 128 * sum(Ks2) * P2
    zb1 = bool(np.all(np.asarray(b1) == 0.0))
    zb2 = bool(np.all(np.asarray(b2) == 0.0))
    key = (Ks, zb1, zb2)
    if key not in _cache:
        _cache.clear()
        _cache[key] = (_build_tform(), _build_l1(Ks, s1len, zb1),
                       _build_l2(Ks2, s2len, zb2))
    ncA, ncB, ncC = _cache[key]
    core_ids = list(range(NCORES))

    # ---- launch A: per-node transform ----
    in_mapsA = [dict(xlocT=pc["xlocT"], W1aug=shared["W1aug"])
                for pc in per_core]
    resA = _run_retry(ncA, in_mapsA, core_ids, TRACE)
    if TRACE and resA.exec_time_ns:
        LAST_EXEC_NS.append(resA.exec_time_ns)

    # host: node payload table [h | ones | a_src] + per-core a_dst
    tab1 = np.empty((N, P1), dtype=np.float32)
    tab1[:, 64:72] = 1.0
    adTs = []
    for k in range(NCORES):
        t1k = resA.results[k]["t1"]                     # [NL, 80]
        sid = per_core[k]["sorted_ids"]
        tab1[sid, :64] = t1k[:NLOC, :64]
        tab1[sid, 72:80] = t1k[:NLOC, 64:72]
        adTs.append(np.ascontiguousarray(
            t1k[:, 72:80].reshape(NT, 128, 8).transpose(1, 0, 2)
            .reshape(128, NT * 8)))

    b1t = np.tile(np.asarray(b1, np.float32)[shared["perm"]][None, :],
                  (128, 1))
    W2blk = np.zeros((128, 36), dtype=np.float32)
    W2blk[0:C1, 0:18] = shared["W2aug"]
    W2blk[C1:128, 18:36] = shared["W2aug"]
    in_mapsB = []
    for k in range(NCORES):
        pc = per_core[k]
        s1 = _build_stream(Ks, pc["srcs"], pc["valid"], tab1, P1, 72, 80)
        in_mapsB.append(dict(s1=s1, adT=adTs[k], b1=b1t, W2blk=W2blk))
    resB = _run_retry(ncB, in_mapsB, core_ids, TRACE)
    if TRACE and resB.exec_time_ns:
        LAST_EXEC_NS.append(resB.exec_time_ns)

    # host: layer-2 payload table + per-core a_dst2
    tab2 = np.zeros((N, P2), dtype=np.float32)
    ad2s = []
    for k in range(NCORES):
        p2k = resB.results[k]["p2"]                     # [NL, 18]
        tab2[per_core[k]["sorted_ids"]] = p2k[:NLOC, :P2]
        ad2s.append(np.ascontiguousarray(p2k[:, 17].reshape(NT, 128).T))

    b2t = np.tile(np.asarray(b2, np.float32)[None, :], (128, 1))
    in_mapsC = []
    for k in range(NCORES):
        pc = per_core[k]
        s2 = _build_stream(Ks2, pc["srcs"], pc["valid"], tab2, P2, 16, 17,
                           paired=True)
        in_mapsC.append(dict(s2=s2, ad2=ad2s[k], b2=b2t))
    resC = _run_retry(ncC, in_mapsC, core_ids, TRACE)
    if TRACE and resC.exec_time_ns:
        LAST_EXEC_NS.append(resC.exec_time_ns)

    out = np.zeros((N, OUT_C), dtype=np.float32)
    for k in range(NCORES):
        out[per_core[k]["sorted_ids"]] = resC.results[k]["out2"][:NLOC]
    return out


# revision 53
# speedup vs baseline: 1.0555x; 1.0555x over previous
"""GAT (2-layer) Trainium2 Bass kernel — streamed-edge, engine-balanced.

Strategy (8 NeuronCores, SPMD, 3 launches):
  - Destination-sharded edge parallelism: core k owns dst nodes
    [12500k, 12500(k+1)), degree-sorted and packed into ELL tiles
    [128 dst x K_t slots] (K_t padded to a multiple of 4).
  - Per-edge source payloads are assembled on the host (pure data
    movement between launches) and DMA-streamed sequentially; no
    device-side gather.
      launch A: per-node transform  [h | a_src | a_dst] = x @ W1aug
      launch B: layer-1 edge pass (softmax + aggregate + ELU + @W2aug)
      launch C: layer-2 edge pass -> final output rows
  - Engine balance in the edge passes: message multiply on gpsimd,
    tree-fold + reduce on DVE, exp/ELU/PSUM-copies on the scalar
    engine, h2@W2aug on the tensor engine.
  - Layer-1 payload rows are [h(64) | ones(8) | a_src(8)] so the
    softmax denominator falls out of the same fold/reduce tree as the
    numerator (features 64:72 of the weighted message are the weights).
  - Layer-2 uses exp(lrelu(x)) == max(exp(x), exp(0.2x)) so the weight
    needs no lrelu op, and processes tiles in pairs (pair-equalized K,
    host-interleaved stream) to amortize per-op overheads; layer-1
    finalize (ELU + @W2aug) is likewise paired, with one 128x128
    transpose and a block-diagonal W2 matmul per pair.
  - Padded ELL slots carry a_src = ASENT so their weight is exactly 0.

kernel(**inputs) -> np.ndarray [100000, 16] float32.
"""
import sys

sys.path.insert(0, "/opt/trn_rl_repo")

import numpy as np
import concourse.bass as bass
import concourse.bacc as bacc
import concourse.tile as tile
from concourse import mybir
from concourse.bass_utils import run_bass_kernel_spmd

AP = bass.AP
F32 = mybir.dt.float32
AF = mybir.ActivationFunctionType
ALU = mybir.AluOpType
AX = mybir.AxisListType

# Problem constants (hardcoded per the harness contract).
N = 100000
E = 1600000
IN_C = 128
HID = 8
HEADS = 8
C1 = HEADS * HID          # 64
OUT_C = 16
NEG_SLOPE = 0.2
NCORES = 8

NLOC = N // NCORES        # 12500 local dst nodes per core
NT = 98                   # node tiles of 128 (98*128 = 12544)
NL = NT * 128             # 12544 padded local nodes
P1 = 80                   # layer-1 payload: h(64) | ones(8) | a_src(8)
P2 = 17                   # layer-2 payload: h2W2(16) | a_src2(1)
ASENT = -30000.0          # sentinel a_src; weight becomes exactly 0
EPS = 1e-16

_cache = {}


# --------------------------------------------------------------------------
# Host-side preprocessing (pure data movement + O(F^2) weight prep)
# --------------------------------------------------------------------------
def _prep(x, edge_index, W1, att_src1, att_dst1, W2, att_src2, att_dst2):
    src = edge_index[0].astype(np.int64)
    dst = edge_index[1].astype(np.int64)

    W1r = W1.reshape(IN_C, HEADS, HID)
    v_src1 = np.einsum("khc,hc->kh", W1r, att_src1).astype(np.float32)
    v_dst1 = np.einsum("khc,hc->kh", W1r, att_dst1).astype(np.float32)
    # h is stored feature-major (column 8b+j = feat b, head j) so both the
    # h-blocks and the ones-block of the edge-pass multiply broadcast the
    # per-head weight along the innermost 8 elements.
    perm = np.arange(C1).reshape(HEADS, HID).T.reshape(-1)
    W1aug = np.concatenate(
        [W1[:, perm], v_src1, v_dst1], axis=1).astype(np.float32)
    v_src2 = (W2 @ att_src2[0]).astype(np.float32)
    v_dst2 = (W2 @ att_dst2[0]).astype(np.float32)
    W2aug = np.concatenate(
        [W2, v_src2[:, None], v_dst2[:, None]],
        axis=1).astype(np.float32)[perm, :]

    order = np.argsort(dst, kind="stable")
    deg = np.bincount(dst, minlength=N).astype(np.int64)
    cum = np.zeros(N + 1, dtype=np.int64)
    np.cumsum(deg, out=cum[1:])

    cores = []
    for k in range(NCORES):
        ids = np.arange(k * NLOC, (k + 1) * NLOC)
        dk = deg[ids]
        sp = np.argsort(-dk, kind="stable")
        cores.append((ids[sp], dk[sp]))

    # global K schedule, padded to a multiple of 4
    K = np.zeros(NT, dtype=np.int64)
    for k in range(NCORES):
        ds = np.zeros(NL, dtype=np.int64)
        ds[:NLOC] = cores[k][1]
        K = np.maximum(K, ds.reshape(NT, 128).max(axis=1))
    K = ((K + 3) // 4) * 4
    K = np.maximum(K, 4)
    Ks = tuple(int(v) for v in K)

    Kmax = int(K.max())
    per_core = []
    for k in range(NCORES):
        sorted_ids, deg_sorted = cores[k]
        dpad = np.zeros(NL, dtype=np.int64)
        dpad[:NLOC] = deg_sorted
        start = np.zeros(NL, dtype=np.int64)
        start[:NLOC] = cum[sorted_ids]
        colr = np.arange(Kmax)
        valid = colr[None, :] < dpad[:, None]              # [NL, Kmax]
        epos = start[:, None] + colr[None, :]
        srcs = np.zeros((NL, Kmax), dtype=np.int64)
        srcs[valid] = src[order[epos[valid]]]

        xlocT = np.zeros((IN_C, NL), dtype=np.float32)
        xlocT[:, :NLOC] = x[sorted_ids].T

        per_core.append(dict(srcs=srcs, valid=valid, xlocT=xlocT,
                             sorted_ids=sorted_ids))

    shared = dict(W1aug=W1aug, W2aug=W2aug, perm=perm)
    return shared, per_core, Ks


def _build_stream(Ks, srcs, valid, tab, width, sent_lo, sent_hi,
                  paired=False):
    """Per-edge payload stream: one [128, K_t, width] block per tile, or
    (paired) one [128, 2, K, width] block per tile pair."""
    def tile_pay(t, kt):
        rows = slice(t * 128, (t + 1) * 128)
        blk = srcs[rows, :kt]                           # [128, kt]
        pay = tab[blk.reshape(-1)].reshape(128, kt, width)
        iv = ~valid[rows, :kt]
        if iv.any():
            pay[iv, sent_lo:sent_hi] = ASENT
        return pay
    parts = []
    if paired:
        for m in range(len(Ks) // 2):
            kt = Ks[2 * m]
            pair = np.stack([tile_pay(2 * m, kt), tile_pay(2 * m + 1, kt)],
                            axis=1)                     # [128, 2, kt, w]
            parts.append(pair.reshape(-1))
    else:
        for t, kt in enumerate(Ks):
            parts.append(tile_pay(t, kt).reshape(-1))
    return np.concatenate(parts)


# --------------------------------------------------------------------------
# Launch A: per-node transform  t1T = [W1 | v_src1 | v_dst1]^T @ xloc
# --------------------------------------------------------------------------
def _build_tform():
    nc = bacc.Bacc("TRN2", target_bir_lowering=False, debug=False,
                   num_devices=NCORES)
    xlocT = nc.dram_tensor("xlocT", [IN_C, NL], F32, kind="ExternalInput").ap()
    W1aug = nc.dram_tensor("W1aug", [IN_C, P1], F32, kind="ExternalInput").ap()
    t1 = nc.dram_tensor("t1", [NL, P1], F32, kind="ExternalOutput").ap()

    with tile.TileContext(nc) as tc:
        with tc.tile_pool(name="const", bufs=1) as constp, \
             tc.tile_pool(name="lhs", bufs=3) as lhsp, \
             tc.tile_pool(name="ps", bufs=2, space="PSUM") as psp, \
             tc.tile_pool(name="cp", bufs=3) as cpp:
            w = constp.tile([IN_C, P1], F32)
            nc.sync.dma_start(out=w[:], in_=W1aug[:, :])
            for B in range((NL + 511) // 512):
                nb = min(4, (NL - 512 * B) // 128)
                lt = lhsp.tile([IN_C, 512], F32, tag="lhs")
                eng = nc.sync if B % 2 == 0 else nc.scalar
                eng.dma_start(
                    out=lt[:, :128 * nb],
                    in_=xlocT[:, 512 * B: 512 * B + 128 * nb])
                ps = psp.tile([128, 4, P1], F32)
                for j in range(nb):
                    nc.tensor.matmul(out=ps[:, j, :],
                                     lhsT=lt[:, 128 * j: 128 * (j + 1)],
                                     rhs=w[:], start=True, stop=True)
                cp = cpp.tile([128, 4, P1], F32, tag="cp")
                nc.scalar.activation(out=cp[:, :nb, :], in_=ps[:, :nb, :],
                                     func=AF.Copy)
                nc.sync.dma_start(
                    out=AP(tensor=t1.tensor, offset=512 * B * P1,
                           ap=[[P1, 128], [128 * P1, nb], [1, P1]]),
                    in_=cp[:, :nb, :])
    nc.compile()
    return nc


# --------------------------------------------------------------------------
# Launch B: layer-1 edge pass + finalize (ELU + @W2aug)
# --------------------------------------------------------------------------
def _build_l1(Ks, s1len, zb1):
    nc = bacc.Bacc("TRN2", target_bir_lowering=False, debug=False,
                   num_devices=NCORES)
    s1 = nc.dram_tensor("s1", [s1len], F32, kind="ExternalInput").ap()
    adT = nc.dram_tensor("adT", [128, NT * 8], F32, kind="ExternalInput").ap()
    b1 = nc.dram_tensor("b1", [128, C1], F32, kind="ExternalInput").ap()
    W2blk = nc.dram_tensor("W2blk", [128, 36], F32, kind="ExternalInput").ap()
    p2 = nc.dram_tensor("p2", [NL, 18], F32, kind="ExternalOutput").ap()

    from concourse.masks import make_identity
    KM = max(Ks)

    with tile.TileContext(nc) as tc:
        with tc.tile_pool(name="const", bufs=1) as constp, \
             tc.tile_pool(name="gp", bufs=5) as gp, \
             tc.tile_pool(name="ep", bufs=5) as ep, \
             tc.tile_pool(name="tp", bufs=5) as tp, \
             tc.tile_pool(name="fp", bufs=4) as fp, \
             tc.tile_pool(name="zp", bufs=4) as zp, \
             tc.tile_pool(name="psT", bufs=3, space="PSUM") as psTp, \
             tc.tile_pool(name="psP", bufs=3, space="PSUM") as psPp:

            adsb = constp.tile([128, NT * 8], F32)
            nc.sync.dma_start(out=adsb[:], in_=adT[:, :])
            b1sb = constp.tile([128, C1], F32)
            nc.sync.dma_start(out=b1sb[:], in_=b1[:, :])
            w2sb = constp.tile([128, 36], F32)
            nc.sync.dma_start(out=w2sb[:], in_=W2blk[:, :])
            ident = constp.tile([128, 128], F32)
            make_identity(nc, ident[:])
            epssb = constp.tile([128, 1], F32)
            nc.vector.memset(epssb[:], EPS)

            off = 0
            aggq = None
            for t in range(NT):
                kt = Ks[t]
                kh = kt // 2
                # one DMA per tile
                G = gp.tile([128, KM, P1], F32, tag="G")
                nc.sync.dma_start(
                    out=G[:, :kt, :],
                    in_=AP(tensor=s1.tensor, offset=off,
                           ap=[[kt * P1, 128], [1, kt * P1]]))
                off += 128 * kt * P1
                go = G[:, :kt, :].offset
                # w = exp(lrelu(a_src + a_dst))   [128, kt, 8]
                ea = ep.tile([128, KM, 8], F32, tag="ea")
                eav = ea[:, :kt, :]
                nc.gpsimd.tensor_tensor(
                    out=eav,
                    in0=AP(tensor=G.tensor, offset=go + 72,
                           ap=[G[:].ap[0], [P1, kt], [1, 8]]),
                    in1=AP(tensor=adsb.tensor,
                           offset=adsb[:].offset + 8 * t,
                           ap=[adsb[:].ap[0], [0, kt], [1, 8]]),
                    op=ALU.add)
                nc.vector.scalar_tensor_tensor(
                    out=eav, in0=eav, scalar=NEG_SLOPE, in1=eav,
                    op0=ALU.mult, op1=ALU.max)
                nc.scalar.activation(out=eav, in_=eav, func=AF.Exp)
                # T = [h|ones] * w  -> [128, kt, 9, 8]  (gpsimd)
                T = tp.tile([128, KM, 9, 8], F32, tag="T")
                Tv = T[:, :kt, :, :]
                nc.gpsimd.tensor_tensor(
                    out=Tv,
                    in0=AP(tensor=G.tensor, offset=go,
                           ap=[G[:].ap[0], [P1, kt], [8, 9], [1, 8]]),
                    in1=AP(tensor=ea.tensor, offset=eav.offset,
                           ap=[ea[:].ap[0], [8, kt], [0, 9], [1, 8]]),
                    op=ALU.mult)
                # fold + reduce over slots -> aggq[q] = [num(64) | den(8)]
                F = fp.tile([128, KM // 2, 9, 8], F32, tag="F")
                Fv = F[:, :kh, :, :]
                nc.vector.tensor_tensor(
                    out=Fv, in0=T[:, :kh, :, :], in1=T[:, kh:kt, :, :],
                    op=ALU.add)
                kq = kh // 2
                F2 = fp.tile([128, KM // 4, 9, 8], F32, tag="F2")
                F2v = F2[:, :kq, :, :]
                nc.vector.tensor_tensor(
                    out=F2v, in0=F[:, :kq, :, :], in1=F[:, kq:kh, :, :],
                    op=ALU.add)
                q = t % 2
                if q == 0:
                    aggq = zp.tile([128, 2, 72], F32, tag="agg")
                nc.vector.tensor_reduce(
                    out=aggq[:, q, :],
                    in_=AP(tensor=F2.tensor, offset=F2v.offset,
                           ap=[F2[:].ap[0], [1, 72], [72, kq]]),
                    axis=AX.X, op=ALU.add)
                if q == 0:
                    continue
                # finalize the pair (t-1, t)
                den = zp.tile([128, 2, 8], F32, tag="den")
                nc.scalar.activation(
                    out=den[:],
                    in_=AP(tensor=aggq.tensor, offset=aggq[:].offset + 64,
                           ap=[aggq[:].ap[0], [72, 2], [1, 8]]),
                    func=AF.Identity, bias=epssb[:, 0:1])
                rec = zp.tile([128, 2, 8], F32, tag="rec")
                nc.vector.reciprocal(rec[:], den[:])
                out1 = zp.tile([128, 2, C1], F32, tag="out1")
                nc.vector.tensor_tensor(
                    out=out1[:],
                    in0=AP(tensor=aggq.tensor, offset=aggq[:].offset,
                           ap=[aggq[:].ap[0], [72, 2], [1, 64]]),
                    in1=AP(tensor=rec.tensor, offset=rec[:].offset,
                           ap=[rec[:].ap[0], [8, 2], [0, 8], [1, 8]]),
                    op=ALU.mult)
                if not zb1:
                    nc.vector.tensor_tensor(
                        out=out1[:], in0=out1[:],
                        in1=AP(tensor=b1sb.tensor, offset=b1sb[:].offset,
                               ap=[b1sb[:].ap[0], [0, 2], [1, C1]]),
                        op=ALU.add)
                # elu = relu(x) + exp(min(x,0)) - 1
                r = zp.tile([128, 2, C1], F32, tag="relu")
                nc.scalar.activation(out=r[:], in_=out1[:], func=AF.Relu)
                mn = zp.tile([128, 2, C1], F32, tag="mn")
                nc.scalar.activation(out=mn[:], in_=out1[:], func=AF.Relu,
                                     scale=-1.0)
                nc.scalar.activation(out=mn[:], in_=mn[:], func=AF.Exp,
                                     scale=-1.0)
                h2 = zp.tile([128, 2, C1], F32, tag="h2")
                nc.vector.scalar_tensor_tensor(
                    out=h2[:], in0=r[:], scalar=-1.0, in1=mn[:],
                    op0=ALU.add, op1=ALU.add)
                # p2 pair = h2 @ blockdiag(W2aug, W2aug); one transpose
                pst = psTp.tile([128, 128], F32)
                nc.tensor.transpose(
                    out=pst[:],
                    in_=AP(tensor=h2.tensor, offset=h2[:].offset,
                           ap=[h2[:].ap[0], [1, 128]]),
                    identity=ident[:])
                h2T = zp.tile([128, 128], F32, tag="h2T")
                nc.scalar.activation(out=h2T[:], in_=pst[:], func=AF.Copy)
                psp = psPp.tile([128, 2, 18], F32)
                nc.tensor.matmul(out=psp[:], lhsT=h2T[:], rhs=w2sb[:],
                                 start=True, stop=True)
                p2sb = zp.tile([128, 2, 18], F32, tag="p2sb")
                nc.scalar.activation(out=p2sb[:], in_=psp[:], func=AF.Copy)
                nc.sync.dma_start(
                    out=AP(tensor=p2.tensor, offset=(t - 1) * 128 * 18,
                           ap=[[18, 128], [128 * 18, 2], [1, 18]]),
                    in_=p2sb[:])
    nc.compile()
    return nc


# --------------------------------------------------------------------------
# Launch C: layer-2 edge pass
# --------------------------------------------------------------------------
def _build_l2(Ks2, s2len, zb2):
    nc = bacc.Bacc("TRN2", target_bir_lowering=False, debug=False,
                   num_devices=NCORES)
    s2 = nc.dram_tensor("s2", [s2len], F32, kind="ExternalInput").ap()
    ad2 = nc.dram_tensor("ad2", [128, NT], F32, kind="ExternalInput").ap()
    b2 = nc.dram_tensor("b2", [128, OUT_C], F32, kind="ExternalInput").ap()
    out2 = nc.dram_tensor("out2", [NL, OUT_C], F32, kind="ExternalOutput").ap()
    KM = max(Ks2)

    with tile.TileContext(nc) as tc:
        with tc.tile_pool(name="const", bufs=1) as constp, \
             tc.tile_pool(name="gp", bufs=5) as gp, \
             tc.tile_pool(name="ep", bufs=4) as ep, \
             tc.tile_pool(name="tp", bufs=4) as tp, \
             tc.tile_pool(name="zp", bufs=4) as zp:

            adsb = constp.tile([128, NT], F32)
            nc.sync.dma_start(out=adsb[:], in_=ad2[:, :])
            b2sb = constp.tile([128, OUT_C], F32)
            nc.sync.dma_start(out=b2sb[:], in_=b2[:, :])
            epssb = constp.tile([128, 1], F32)
            nc.vector.memset(epssb[:], EPS)

            off = 0
            for m in range(NT // 2):
                kt = Ks2[2 * m]
                # one DMA per tile pair (both tiles share kt)
                G = gp.tile([128, 2, KM, P2], F32, tag="G")
                Gv = G[:, :, :kt, :]
                eng = nc.sync if m % 2 == 0 else nc.scalar
                eng.dma_start(
                    out=Gv,
                    in_=AP(tensor=s2.tensor, offset=off,
                           ap=[[2 * kt * P2, 128], [1, 2 * kt * P2]]))
                off += 128 * 2 * kt * P2
                go = Gv.offset
                gq = KM * P2          # q stride inside G
                # e = a_src + a_dst   [128, 2, kt]
                e = ep.tile([128, 2, KM], F32, tag="e")
                ev = e[:, :, :kt]
                nc.gpsimd.tensor_tensor(
                    out=ev,
                    in0=AP(tensor=G.tensor, offset=go + 16,
                           ap=[G[:].ap[0], [gq, 2], [P2, kt]]),
                    in1=AP(tensor=adsb.tensor,
                           offset=adsb[:].offset + 2 * m,
                           ap=[adsb[:].ap[0], [1, 2], [0, kt]]),
                    op=ALU.add)
                # w = exp(lrelu(e)) = max(exp(e), exp(0.2 e))
                e2 = ep.tile([128, 2, KM], F32, tag="e2")
                e2v = e2[:, :, :kt]
                nc.scalar.activation(out=e2v, in_=ev, func=AF.Exp,
                                     scale=NEG_SLOPE)
                nc.scalar.activation(out=ev, in_=ev, func=AF.Exp)
                w = ep.tile([128, 2, KM], F32, tag="w")
                wv = w[:, :, :kt]
                nc.vector.tensor_tensor(out=wv, in0=ev, in1=e2v, op=ALU.max)
                den = zp.tile([128, 2], F32, tag="den")
                nc.vector.tensor_reduce(
                    out=den[:],
                    in_=AP(tensor=w.tensor, offset=wv.offset,
                           ap=[w[:].ap[0], [KM, 2], [1, kt]]),
                    axis=AX.X, op=ALU.add)
                rec = zp.tile([128, 2], F32, tag="rec")
                nc.scalar.activation(out=den[:], in_=den[:],
                                     func=AF.Identity, bias=epssb[:, 0:1])
                nc.vector.reciprocal(rec[:], den[:])
                # T = h2w2 * w   (gpsimd)
                T = tp.tile([128, 2, KM, OUT_C], F32, tag="T")
                Tv = T[:, :, :kt, :]
                nc.gpsimd.tensor_tensor(
                    out=Tv,
                    in0=AP(tensor=G.tensor, offset=go,
                           ap=[G[:].ap[0], [gq, 2], [P2, kt], [1, OUT_C]]),
                    in1=AP(tensor=w.tensor, offset=wv.offset,
                           ap=[w[:].ap[0], [KM, 2], [1, kt], [0, OUT_C]]),
                    op=ALU.mult)
                num = zp.tile([128, 2, OUT_C], F32, tag="num")
                nc.vector.tensor_reduce(
                    out=num[:],
                    in_=AP(tensor=T.tensor, offset=Tv.offset,
                           ap=[T[:].ap[0], [KM * OUT_C, 2], [1, OUT_C],
                               [OUT_C, kt]]),
                    axis=AX.X, op=ALU.add)
                o = zp.tile([128, 2, OUT_C], F32, tag="o")
                nc.vector.tensor_tensor(
                    out=o[:], in0=num[:],
                    in1=AP(tensor=rec.tensor, offset=rec[:].offset,
                           ap=[rec[:].ap[0], [1, 2], [0, OUT_C]]),
                    op=ALU.mult)
                if not zb2:
                    nc.vector.tensor_tensor(
                        out=o[:], in0=o[:],
                        in1=AP(tensor=b2sb.tensor, offset=b2sb[:].offset,
                               ap=[b2sb[:].ap[0], [0, 2], [1, OUT_C]]),
                        op=ALU.add)
                nc.sync.dma_start(
                    out=AP(tensor=out2.tensor, offset=2 * m * 128 * OUT_C,
                           ap=[[OUT_C, 128], [128 * OUT_C, 2], [1, OUT_C]]),
                    in_=o[:])
    nc.compile()
    return nc


# --------------------------------------------------------------------------
# Entry point
# --------------------------------------------------------------------------
TRACE = False
LAST_EXEC_NS = []


def _run_retry(nc, in_maps, core_ids, trace):
    import time as _time
    last = None
    for attempt in range(3):
        try:
            return run_bass_kernel_spmd(nc, in_maps, core_ids, trace=trace)
        except Exception as e:  # transient NRT_EXEC_UNIT_UNRECOVERABLE
            last = e
            _time.sleep(10)
    raise last


def kernel(x, edge_index, W1, b1, att_src1, att_dst1, W2, b2, att_src2,
           att_dst2):
    global LAST_EXEC_NS
    LAST_EXEC_NS = []
    x = np.asarray(x, dtype=np.float32)
    edge_index = np.asarray(edge_index)
    shared, per_core, Ks = _prep(
        x, edge_index, np.asarray(W1), np.asarray(att_src1),
        np.asarray(att_dst1), np.asarray(W2), np.asarray(att_src2),
        np.asarray(att_dst2))

    s1len = 128 * sum(Ks) * P1
    Ks2 = tuple(max(Ks[2 * m], Ks[2 * m + 1]) for m in range(NT // 2)
                for _ in range(2))
    s2len = 128 * sum(Ks2) * P2
    zb1 = bool(np.all(np.asarray(b1) == 0.0))
    zb2 = bool(np.all(np.asarray(b2) == 0.0))
    key = (Ks, zb1, zb2)
    if key not in _cache:
        _cache.clear()
        _cache[key] = (_build_tform(), _build_l1(Ks, s1len, zb1),
                       _build_l2(Ks2, s2len, zb2))
    ncA, ncB, ncC = _cache[key]
    core_ids = list(range(NCORES))

    # ---- launch A: per-node transform ----
    in_mapsA = [dict(xlocT=pc["xlocT"], W1aug=shared["W1aug"])
                for pc in per_core]
    resA = _run_retry(ncA, in_mapsA, core_ids, TRACE)
    if TRACE and resA.exec_time_ns:
        LAST_EXEC_NS.append(resA.exec_time_ns)

    # host: node payload table [h | ones | a_src] + per-core a_dst
    tab1 = np.empty((N, P1), dtype=np.float32)
    tab1[:, 64:72] = 1.0
    adTs = []
    for k in range(NCORES):
        t1k = resA.results[k]["t1"]                     # [NL, 80]
        sid = per_core[k]["sorted_ids"]
        tab1[sid, :64] = t1k[:NLOC, :64]
        tab1[sid, 72:80] = t1k[:NLOC, 64:72]
        adTs.append(np.ascontiguousarray(
            t1k[:, 72:80].reshape(NT, 128, 8).transpose(1, 0, 2)
            .reshape(128, NT * 8)))

    b1t = np.tile(np.asarray(b1, np.float32)[shared["perm"]][None, :],
                  (128, 1))
    W2blk = np.zeros((128, 36), dtype=np.float32)
    W2blk[0:C1, 0:18] = shared["W2aug"]
    W2blk[C1:128, 18:36] = shared["W2aug"]
    in_mapsB = []
    for k in range(NCORES):
        pc = per_core[k]
        s1 = _build_stream(Ks, pc["srcs"], pc["valid"], tab1, P1, 72, 80)
        in_mapsB.append(dict(s1=s1, adT=adTs[k], b1=b1t, W2blk=W2blk))
    resB = _run_retry(ncB, in_mapsB, core_ids, TRACE)
    if TRACE and resB.exec_time_ns:
        LAST_EXEC_NS.append(resB.exec_time_ns)

    # host: layer-2 payload table + per-core a_dst2
    tab2 = np.zeros((N, P2), dtype=np.float32)
    ad2s = []
    for k in range(NCORES):
        p2k = resB.results[k]["p2"]                     # [NL, 18]
        tab2[per_core[k]["sorted_ids"]] = p2k[:NLOC, :P2]
        ad2s.append(np.ascontiguousarray(p2k[:, 17].reshape(NT, 128).T))

    b2t = np.tile(np.asarray(b2, np.float32)[None, :], (128, 1))
    in_mapsC = []
    for k in range(NCORES):
        pc = per_core[k]
        s2 = _build_stream(Ks2, pc["srcs"], pc["valid"], tab2, P2, 16, 17,
                           paired=True)
        in_mapsC.append(dict(s2=s2, ad2=ad2s[k], b2=b2t))
    resC = _run_retry(ncC, in_mapsC, core_ids, TRACE)
    if TRACE and resC.exec_time_ns:
        LAST_EXEC_NS.append(resC.exec_time_ns)

    out = np.zeros((N, OUT_C), dtype=np.float32)
    for k in range(NCORES):
        out[per_core[k]["sorted_ids"]] = resC.results[k]["out2"][:NLOC]
    return out
